# revision 2
# baseline (speedup 1.0000x reference)
"""GCNCritic forward kernel for Trainium2 (Bass/Tile), 8-core data-parallel.

Math collapse (same as baseline): fully-connected 16-node graphs with
self-loops make each GCN layer's output constant across a sample's nodes, so
the GCN stack reduces to per-sample matmuls on the per-sample sum of
x = relu(obs @ W_pre + b_pre):

    xsum = sum_nodes(relu(obs @ W_pre + b_pre))           # [B, HID]
    x1 = relu(xsum/16 @ W_gcn0 + b_gcn0)                  # [B, HID]
    x2 = relu(x1 @ W_gcn1 + b_gcn1)                       # [B, HID]
    g  = relu(x2 @ W_post + b_post)                       # [B, GE]
    gz = g @ W1[:GE] + b1                                 # [B, F1]
    loc = relu(obs @ W_loc + b_loc)                       # [B*n, LE]
    z1 = relu(loc @ W1[GE:] + gz[sample])                 # [B*n, F1]
    z2 = relu(z1 @ W2 + b2)                               # [B*n, F2]
    q  = z2 @ W3  (+ b3 added on host)                    # [B*n, 8]

v2 vs baseline:
  * All matmul operands fp16 (same 1 cyc/row PE rate as float32r here, but
    halves obs+weight DMA; rel err ~1e-3 << 2e-2).
  * q = z2 @ W3 operand-swapped: z2^T node-blocks stationary, W3 [128,8]
    streams -> 8-row matmuls (16384 -> 1024 PE cycles) and node-major
    output [R, 8], the natural layout (b3 added on host).
  * Software-pipelined C phase: z2(t-1) runs kb-major so it consumes
    z1(t-1) m-blocks as their DVE add+relu evacuations land; q(t-2) and
    the deferred loc/x tiles fill remaining PE slots.
  * Evacuations balanced: ACT (x relu, z2 relu, B chain), DVE (z1 add,
    z1 relu in 4x all-SBUF-fp16 mode, gz, q, loc m0), Pool (xsum reduce,
    loc m1).
  * Output DMA split per 2 tiles to shorten the trailing-DMA tail.
"""

import numpy as np

import concourse.bass as bass
import concourse.mybir as mybir
import concourse.tile as tile
from concourse.bass import ts
from concourse.bass_utils import run_bass_kernel_spmd

OBS = 128
N_AGENT = 16
HID = 128
GE = 256
LE = 256
F1 = 512
F2 = 512
NA = 8
B = 2048
NCORES = 8
BS = B // NCORES            # 256 samples per core
R = BS * N_AGENT            # 4096 rows (nodes) per core
RT = 512                    # rows per tile
NT = R // RT                # 8 row tiles
SPT = RT // N_AGENT         # 32 samples per row tile

F32 = mybir.dt.float32
F16 = mybir.dt.float16
RELU = mybir.ActivationFunctionType.Relu
AL = mybir.AluOpType

NB_F32 = 15
C_BIAS = 0
C_WPRE = 2 * NB_F32          # 30
C_WLOC = C_WPRE + 128        # 158
C_WG0 = C_WLOC + 256         # 414
C_WG1 = C_WG0 + 128          # 542
C_WPOST = C_WG1 + 128        # 670
C_W1A = C_WPOST + 256        # 926   (kb-major: kb*512 + m*128)
C_W1B = C_W1A + 1024         # 1950
C_W2 = C_W1B + 1024          # 2974  (m-major: m*512 + kb*128)
C_W3 = C_W2 + 2048           # 5022  (kb*8)
COLS = C_W3 + 32             # 5054

B_PRE = 0
B_G0 = 1
B_G1 = 2
B_POST = 3
B_LOC = 5
B_1 = 7
B_2 = 11


def _pack_weights(i):
    pk = np.zeros((128, COLS), np.float16)
    f16 = lambda a: np.asarray(a, np.float32).astype(np.float16)
    pk[:, C_WPRE:C_WPRE + 128] = f16(i["W_pre"])
    pk[:, C_WLOC:C_WLOC + 256] = f16(i["W_loc"])
    pk[:, C_WG0:C_WG0 + 128] = f16(i["W_gcn"][0])
    pk[:, C_WG1:C_WG1 + 128] = f16(i["W_gcn"][1])
    pk[:, C_WPOST:C_WPOST + 256] = f16(i["W_post"])
    for kb in range(2):
        for m in range(4):
            pk[:, C_W1A + kb * 512 + m * 128:C_W1A + kb * 512 + (m + 1) * 128] = \
                f16(i["W1"][kb * 128:(kb + 1) * 128, m * 128:(m + 1) * 128])
            pk[:, C_W1B + kb * 512 + m * 128:C_W1B + kb * 512 + (m + 1) * 128] = \
                f16(i["W1"][GE + kb * 128:GE + (kb + 1) * 128, m * 128:(m + 1) * 128])
    for kb in range(4):
        for m in range(4):
            pk[:, C_W2 + m * 512 + kb * 128:C_W2 + m * 512 + (kb + 1) * 128] = \
                f16(i["W2"][kb * 128:(kb + 1) * 128, m * 128:(m + 1) * 128])
        pk[:, C_W3 + kb * 8:C_W3 + (kb + 1) * 8] = f16(i["W3"][kb * 128:(kb + 1) * 128])
    bias = np.zeros((128, NB_F32), np.float32)
    bias[:, B_PRE] = i["b_pre"]
    bias[:, B_G0] = i["b_gcn"][0]
    bias[:, B_G1] = i["b_gcn"][1]
    bias[:, B_POST:B_POST + 2] = i["b_post"].reshape(2, 128).T
    bias[:, B_LOC:B_LOC + 2] = i["b_loc"].reshape(2, 128).T
    bias[:, B_1:B_1 + 4] = i["b1"].reshape(4, 128).T
    bias[:, B_2:B_2 + 4] = i["b2"].reshape(4, 128).T
    pk[:, C_BIAS:C_BIAS + 2 * NB_F32] = bias.view(np.float16)
    return pk


def _build():
    nc = bass.Bass("TRN2", target_bir_lowering=False, debug=False)

    obs_h = nc.dram_tensor("obs", [OBS, R], F16, kind="ExternalInput")
    wpack_h = nc.dram_tensor("wpack", [128, COLS], F16, kind="ExternalInput")
    out_h = nc.dram_tensor("out", [R, NA], F32, kind="ExternalOutput")

    with tile.TileContext(nc) as tc:
        with (
            tc.tile_pool(name="consts", bufs=1) as consts,
            tc.tile_pool(name="persist", bufs=1) as persist,
            tc.tile_pool(name="work", bufs=2) as work,
            tc.tile_pool(name="ps", bufs=1, space="PSUM") as psp,
        ):
            wp = consts.tile([128, COLS], F16, tag="wp")

            def bias(i):
                return wp[:, C_BIAS + 2 * i:C_BIAS + 2 * i + 2].bitcast(F32)

            def wsl(c):
                return wp[:, c:c + 128]

            locT = persist.tile([128, 2, NT, RT], F16, tag="locT")
            xsum = persist.tile([128, BS], F16, tag="xsum")
            g = persist.tile([128, 2, BS], F16, tag="g")
            gz = persist.tile([128, 4, BS], F32, tag="gz")
            qacc = persist.tile([128, 4 * NT, NA], F32, tag="qacc")

            # ---- DMAs ordered by first consumer; the head-critical ones go
            # through Pool SWDGE (tiny seq cost, bypasses the HWDGE queue) ----
            obsTs = [work.tile([128, RT], F16, tag="obsT", bufs=8, name="obsT")
                     for _ in range(NT)]
            nc.sync.dma_start(obsTs[0], obs_h[:, ts(0, RT)])
            nc.scalar.dma_start(wp[:, :C_WLOC], wpack_h[:, :C_WLOC])
            nc.sync.dma_start(wp[:, C_WLOC:C_W1A], wpack_h[:, C_WLOC:C_W1A])
            for t in range(1, 4):
                nc.sync.dma_start(obsTs[t], obs_h[:, ts(t, RT)])
            nc.sync.dma_start(wp[:, C_W1A:C_W2], wpack_h[:, C_W1A:C_W2])
            for t in range(4, 6):
                nc.sync.dma_start(obsTs[t], obs_h[:, ts(t, RT)])
            nc.sync.dma_start(wp[:, C_W2:C_W2 + 1024], wpack_h[:, C_W2:C_W2 + 1024])
            for t in range(6, 8):
                nc.sync.dma_start(obsTs[t], obs_h[:, ts(t, RT)])
            nc.sync.dma_start(wp[:, C_W2 + 1024:], wpack_h[:, C_W2 + 1024:])

            def ps_ax():
                return psp.tile([128, RT], F32, tag="ax", bufs=1, name="ax")

            def warmup():
                # Ramp the PE to full p-state during the head DMA wait:
                # dummy matmuls on scratch data keep the PE continuously
                # busy until obs arrives, so real matmuls run at 2.4 GHz.
                scrA = work.tile([128, 128], F16, tag="scrA", bufs=1,
                                 name="scrA")
                scrB = work.tile([128, RT], F16, tag="scrB", bufs=1,
                                 name="scrB")
                nc.vector.memset(scrA, 0.0)
                nc.gpsimd.memset(scrB, 0.0)
                dps = ps_ax()
                for _ in range(6):
                    nc.tensor.matmul(dps, scrA, scrB, start=True, stop=True)
                nc.vector.tensor_copy(scrB, dps)

            def ps_z1(nm="z1p"):
                return psp.tile([128, RT], F32, tag="z1p", bufs=3, name=nm)

            # ---- X(t): x-path for one tile -> xsum slice ----
            def X(t):
                if t < 4:
                    # preloop: z2p banks are free until iter 1
                    ps = psp.tile([128, 2, RT], F32, tag="z2p", bufs=2,
                                  name="xp2")[:, 0, :]
                elif t % 2 == 0:
                    ps = ps_ax()
                else:
                    ps = ps_z1("xp")
                nc.tensor.matmul(ps, wsl(C_WPRE), obsTs[t], start=True, stop=True)
                xT = work.tile([128, RT], F16, tag="xT", bufs=2, name="xT")
                nc.scalar.activation(xT, ps, RELU, bias=bias(B_PRE))
                with nc.allow_low_precision(reason="fp16 out; 16-elem sum"):
                    nc.vector.tensor_reduce(
                        xsum[:, ts(t, SPT)],
                        xT.rearrange("p (s k) -> p s k", k=N_AGENT),
                        axis=mybir.AxisListType.X,
                        op=AL.add,
                    )

            # ---- LOC(t): loc tile -> locT[:, :, t, :] ----
            def LOC(t):
                for m in range(2):
                    ps = ps_z1("locp")
                    nc.tensor.matmul(ps, wsl(C_WLOC + m * 128), obsTs[t],
                                     start=True, stop=True)
                    nc.scalar.activation(
                        locT[:, m, t, :], ps, RELU, bias=bias(B_LOC + m))

            # ---- B steps over a sample slice (start s0, width w) ----
            def B_x1(s0, w):
                ps = ps_ax()
                nc.tensor.matmul(ps[:, :w], wsl(C_WG0), xsum[:, s0:s0 + w],
                                 start=True, stop=True)
                x1 = work.tile([128, 128], F16, tag="x1", bufs=2, name="x1")
                nc.scalar.activation(x1[:, :w], ps[:, :w], RELU, bias=bias(B_G0),
                                     scale=1.0 / N_AGENT)
                return x1[:, :w]

            def B_x2(x1, w):
                ps = ps_ax()
                nc.tensor.matmul(ps[:, :w], wsl(C_WG1), x1, start=True, stop=True)
                x2 = work.tile([128, 128], F16, tag="x2", bufs=2, name="x2")
                nc.scalar.activation(x2[:, :w], ps[:, :w], RELU, bias=bias(B_G1))
                return x2[:, :w]

            def B_g(x2, s0, w):
                ps = ps_z1("gp")
                for m in range(2):
                    nc.tensor.matmul(ps[:, ts(m, 256)][:, :w], wsl(C_WPOST + m * 128),
                                     x2, start=True, stop=True)
                    nc.scalar.activation(g[:, m, s0:s0 + w], ps[:, ts(m, 256)][:, :w],
                                         RELU, bias=bias(B_POST + m))

            def B_gz(s0, w):
                for mm in range(2):
                    ps = ps_z1("gzp")
                    for m in (2 * mm, 2 * mm + 1):
                        off = (m % 2) * 256
                        for kb in range(2):
                            nc.tensor.matmul(
                                ps[:, off:off + w],
                                wsl(C_W1A + kb * 512 + m * 128),
                                g[:, kb, s0:s0 + w],
                                start=(kb == 0), stop=(kb == 1),
                            )
                        nc.vector.tensor_scalar_add(
                            gz[:, m, s0:s0 + w], ps[:, off:off + w], bias(B_1 + m)
                        )

            # ---- C sections ----
            z1s = {}
            z2s = {}

            def z1_sec(t):
                z1 = work.tile([128, 4, RT], F16, tag="z1", bufs=3, name="z1")
                z1s[t] = z1
                for m in range(4):
                    ps = ps_z1()
                    for kb in range(2):
                        nc.tensor.matmul(
                            ps, wsl(C_W1B + kb * 512 + m * 128), locT[:, kb, t, :],
                            start=(kb == 0), stop=(kb == 1),
                        )
                    z1t = work.tile([128, RT], F16, tag="z1t", bufs=3, name="z1t")
                    nc.vector.tensor_add(
                        z1t.rearrange("p (s k) -> p s k", k=N_AGENT),
                        ps.rearrange("p (s k) -> p s k", k=N_AGENT),
                        gz[:, m, ts(t, SPT)][:, :, None].to_broadcast(
                            [128, SPT, N_AGENT]),
                    )
                    # last tile: keep DVE clear for the z2(7) evacuations
                    if m % 2 == 0 and t < 7:
                        nc.vector.tensor_scalar(z1[:, m, :], z1t, 0.0, 0.0,
                                                op0=AL.add, op1=AL.max)
                    else:
                        nc.gpsimd.tensor_scalar(z1[:, m, :], z1t, 0.0, 0.0,
                                                op0=AL.add, op1=AL.max)

            def z2_sec(t, spread=False):
                z1 = z1s.pop(t)
                z2 = work.tile([128, 4, RT], F16, tag="z2", bufs=2, name="z2")
                z2s[t] = z2
                pss = [psp.tile([128, 2, RT], F32, tag="z2p", bufs=2, name="z2p")
                       for _ in range(2)]
                for kb in range(4):
                    for m in range(4):
                        nc.tensor.matmul(
                            pss[m // 2][:, m % 2, :],
                            wsl(C_W2 + m * 512 + kb * 128), z1[:, kb, :],
                            start=(kb == 0), stop=(kb == 3),
                        )
                for m in range(4):
                    src = pss[m // 2][:, m % 2, :]
                    if spread and m % 2 == 1:
                        nc.vector.tensor_scalar(z2[:, m, :], src, bias(B_2 + m),
                                                0.0, op0=AL.add, op1=AL.max)
                    else:
                        nc.scalar.activation(z2[:, m, :], src, RELU,
                                             bias=bias(B_2 + m))

            def q_sec(t):
                z2 = z2s.pop(t)
                qp = ps_z1("qp") if t == 7 else ps_ax()
                qv = qp[:, :4 * NA].rearrange("p (nb a) -> p nb a", a=NA)
                for nb in range(4):
                    for kb in range(4):
                        nc.tensor.matmul(
                            qv[:, nb, :], z2[:, kb, ts(nb, 128)],
                            wp[:, C_W3 + kb * 8:C_W3 + (kb + 1) * 8],
                            start=(kb == 0), stop=(kb == 3),
                        )
                nc.vector.tensor_copy(qacc[:, ts(t, 4), :], qv)
                if t % 2 == 1 and t < 6:
                    qt = t // 2
                    nc.sync.dma_start(
                        out_h[ts(qt, 2 * RT), :].rearrange("(nb p) a -> p nb a",
                                                           p=128),
                        qacc[:, ts(qt, 8), :],
                    )
                elif t >= 6:
                    nc.sync.dma_start(
                        out_h[ts(t, RT), :].rearrange("(nb p) a -> p nb a", p=128),
                        qacc[:, ts(t, 4), :],
                    )

            # ---- schedule ----
            warmup()
            # tile-0 B chain first (32 samples) so C can start early
            X(0)
            x1 = B_x1(0, SPT)
            X(1)
            x2 = B_x2(x1, SPT)
            X(2)
            B_g(x2, 0, SPT)
            LOC(0)
            B_gz(0, SPT)
            X(3)
            x1 = B_x1(SPT, SPT)
            LOC(1)
            x2 = B_x2(x1, SPT)
            # iter 0
            z1_sec(0)
            B_g(x2, SPT, SPT); B_gz(SPT, SPT)
            LOC(2); X(4)
            x1 = B_x1(64, 64)
            # iter 1
            z2_sec(0)
            x2 = B_x2(x1, 64)
            z1_sec(1); LOC(3); X(5); X(6)
            B_g(x2, 64, 64); B_gz(64, 64)
            # iter 2
            z2_sec(1); q_sec(0); z1_sec(2); LOC(4); X(7)
            x1 = B_x1(128, 128)
            # iter 3
            z2_sec(2)
            x2 = B_x2(x1, 128)
            q_sec(1); z1_sec(3); LOC(5)
            B_g(x2, 128, 128); B_gz(128, 128)
            # iter 4
            z2_sec(3); q_sec(2); z1_sec(4); LOC(6)
            # iter 5
            z2_sec(4); q_sec(3); z1_sec(5); LOC(7)
            # iter 6
            z2_sec(5); q_sec(4); z1_sec(6)
            # iter 7 (z2(6) evacs stay on ACT: DVE must stay clear
            # for z1(7)'s adds, which gate the flush z2(7) matmuls)
            z2_sec(6); q_sec(5); z1_sec(7)
            # flush
            z2_sec(7, spread=True); q_sec(6); q_sec(7)

    _split_waits(nc)
    return nc


def _split_waits(nc):
    # walrus accepts only one sync-wait per instruction in this build; move
    # extra waits onto same-engine sequencer nops placed immediately before
    # the instruction (program order on the engine's queue, so semantics are
    # identical).
    for blk in nc.m.functions[0].blocks:
        new = []
        for inst in blk.instructions:
            if inst.sync_info is not None:
                w = list(inst.sync_info.on_wait)
                if len(w) > 1:
                    for wx in w[:-1]:
                        new.append(
                            mybir.InstNoOp(
                                name=nc.get_next_instruction_name(),
                                engine=inst.engine,
                                sync_info=mybir.SyncInfo(
                                    on_wait=[wx], on_update=[]
                                ),
                                bass_nofuse=True,
                            )
                        )
                    inst.sync_info.on_wait = [w[-1]]
            new.append(inst)
        blk.instructions[:] = new


_CACHE = {}


def _get_nc():
    if "nc" not in _CACHE:
        _CACHE["nc"] = _build()
    return _CACHE["nc"]


def kernel(trace=False, **inputs):
    obs_j = np.ascontiguousarray(np.asarray(inputs["obs_j"], dtype=np.float32))
    np_in = {
        k: np.asarray(v, dtype=np.float32)
        for k, v in inputs.items()
        if k != "obs_j"
    }
    pack = np.ascontiguousarray(_pack_weights(np_in))
    nc = _get_nc()
    in_maps = []
    for c in range(NCORES):
        in_maps.append({
            "obs": np.ascontiguousarray(
                obs_j[c * BS:(c + 1) * BS].reshape(R, OBS).T.astype(np.float16)),
            "wpack": pack,
        })
    res = run_bass_kernel_spmd(
        nc, in_maps, core_ids=list(range(NCORES)), trace=trace
    )
    out = np.concatenate([r["out"] for r in res.results], axis=0)  # [B*n, NA]
    q = out.reshape(B, N_AGENT, NA) + np_in["b3"]
    if trace:
        return q, res
    return q


# revision 4
# speedup vs baseline: 1.0278x; 1.0278x over previous
"""GCNCritic forward kernel for Trainium2 (Bass/Tile), 8-core data-parallel.

Math collapse (same as baseline): fully-connected 16-node graphs with
self-loops make each GCN layer's output constant across a sample's nodes, so
the GCN stack reduces to per-sample matmuls on the per-sample sum of
x = relu(obs @ W_pre + b_pre):

    xsum = sum_nodes(relu(obs @ W_pre + b_pre))           # [B, HID]
    x1 = relu(xsum/16 @ W_gcn0 + b_gcn0)                  # [B, HID]
    x2 = relu(x1 @ W_gcn1 + b_gcn1)                       # [B, HID]
    g  = relu(x2 @ W_post + b_post)                       # [B, GE]
    gz = g @ W1[:GE] + b1                                 # [B, F1]
    loc = relu(obs @ W_loc + b_loc)                       # [B*n, LE]
    z1 = relu(loc @ W1[GE:] + gz[sample])                 # [B*n, F1]
    z2 = relu(z1 @ W2 + b2)                               # [B*n, F2]
    q  = z2 @ W3  (+ b3 added on host)                    # [B*n, 8]

v2 vs baseline:
  * All matmul operands fp16 (same 1 cyc/row PE rate as float32r here, but
    halves obs+weight DMA; rel err ~1e-3 << 2e-2).
  * q = z2 @ W3 operand-swapped: z2^T node-blocks stationary, W3 [128,8]
    streams -> 8-row matmuls (16384 -> 1024 PE cycles) and node-major
    output [R, 8], the natural layout (b3 added on host).
  * Software-pipelined C phase: z2(t-1) runs kb-major so it consumes
    z1(t-1) m-blocks as their DVE add+relu evacuations land; q(t-2) and
    the deferred loc/x tiles fill remaining PE slots.
  * Evacuations balanced: ACT (x relu, z2 relu, B chain), DVE (z1 add,
    z1 relu in 4x all-SBUF-fp16 mode, gz, q, loc m0), Pool (xsum reduce,
    loc m1).
  * Output DMA split per 2 tiles to shorten the trailing-DMA tail.
"""

import ml_dtypes
import numpy as np

import concourse.bass as bass
import concourse.mybir as mybir
import concourse.tile as tile
from concourse.bass import ts
from concourse.bass_utils import run_bass_kernel_spmd

OBS = 128
N_AGENT = 16
HID = 128
GE = 256
LE = 256
F1 = 512
F2 = 512
NA = 8
B = 2048
NCORES = 8
BS = B // NCORES            # 256 samples per core
R = BS * N_AGENT            # 4096 rows (nodes) per core
RT = 512                    # rows per tile
NT = R // RT                # 8 row tiles
SPT = RT // N_AGENT         # 32 samples per row tile

F32 = mybir.dt.float32
F16 = mybir.dt.float16
E4 = mybir.dt.float8e4
FP8 = ml_dtypes.float8_e4m3
DR = mybir.MatmulPerfMode.DoubleRow
RELU = mybir.ActivationFunctionType.Relu
AL = mybir.AluOpType

NB_F32 = 15
C_BIAS = 0
# fp8 section (stored as fp16 columns, bitcast to fp8e4 on device):
# [0:256)    W_pre*64 duplicated over ko            (x DoubleRow lhsT)
# [256:768)  W_loc_hi*64 dup over ko, m-major       (loc DR term 1)
# [768:1280) W_loc_lo*64 (ko=0) + zeros (ko=1)      (loc DR term 2)
C_F8 = 2 * NB_F32            # 30
NF8 = 640                    # fp16 cols = 1280 fp8 cols
WSCALE = 64.0
C_WG0 = C_F8 + NF8           # 670
C_WG1 = C_WG0 + 128          # 798
C_WPOST = C_WG1 + 128        # 926
C_W1A = C_WPOST + 256        # 1182  (kb-major: kb*512 + m*128)
C_W1B = C_W1A + 1024         # 2206
C_W2 = C_W1B + 1024          # 3230  (m-major: m*512 + kb*128)
C_W3 = C_W2 + 2048           # 5278  (kb*8)
COLS = C_W3 + 32             # 5310

B_PRE = 0
B_G0 = 1
B_G1 = 2
B_POST = 3
B_LOC = 5
B_1 = 7
B_2 = 11


def _pack_weights(i):
    pk = np.zeros((128, COLS), np.float16)
    f16 = lambda a: np.asarray(a, np.float32).astype(np.float16)
    f8 = np.zeros((128, 2 * NF8), FP8)
    wpre8 = (np.asarray(i["W_pre"], np.float32) * WSCALE).astype(FP8)
    f8[:, 0:128] = wpre8
    f8[:, 128:256] = wpre8
    wloc = np.asarray(i["W_loc"], np.float32) * WSCALE
    whi = wloc.astype(FP8)
    wlo = (wloc - whi.astype(np.float32)).astype(FP8)
    f8[:, 256:512] = whi
    f8[:, 512:768] = whi
    f8[:, 768:1024] = wlo
    pk[:, C_F8:C_F8 + NF8] = f8.view(np.float16)
    pk[:, C_WG0:C_WG0 + 128] = f16(np.asarray(i["W_gcn"][0]) / N_AGENT)
    pk[:, C_WG1:C_WG1 + 128] = f16(i["W_gcn"][1])
    pk[:, C_WPOST:C_WPOST + 256] = f16(i["W_post"])
    for kb in range(2):
        for m in range(4):
            pk[:, C_W1A + kb * 512 + m * 128:C_W1A + kb * 512 + (m + 1) * 128] = \
                f16(i["W1"][kb * 128:(kb + 1) * 128, m * 128:(m + 1) * 128])
            pk[:, C_W1B + kb * 512 + m * 128:C_W1B + kb * 512 + (m + 1) * 128] = \
                f16(i["W1"][GE + kb * 128:GE + (kb + 1) * 128, m * 128:(m + 1) * 128])
    for kb in range(4):
        for m in range(4):
            pk[:, C_W2 + m * 512 + kb * 128:C_W2 + m * 512 + (kb + 1) * 128] = \
                f16(i["W2"][kb * 128:(kb + 1) * 128, m * 128:(m + 1) * 128])
        pk[:, C_W3 + kb * 8:C_W3 + (kb + 1) * 8] = f16(i["W3"][kb * 128:(kb + 1) * 128])
    bias = np.zeros((128, NB_F32), np.float32)
    bias[:, B_PRE] = i["b_pre"]
    bias[:, B_G0] = i["b_gcn"][0]
    bias[:, B_G1] = i["b_gcn"][1]
    bias[:, B_POST:B_POST + 2] = i["b_post"].reshape(2, 128).T
    bias[:, B_LOC:B_LOC + 2] = i["b_loc"].reshape(2, 128).T
    bias[:, B_1:B_1 + 4] = i["b1"].reshape(4, 128).T
    bias[:, B_2:B_2 + 4] = i["b2"].reshape(4, 128).T
    pk[:, C_BIAS:C_BIAS + 2 * NB_F32] = bias.view(np.float16)
    return pk


def _build():
    nc = bass.Bass("TRN2", target_bir_lowering=False, debug=False)

    obs_h = nc.dram_tensor("obs", [OBS, 2, R], E4, kind="ExternalInput")
    wpack_h = nc.dram_tensor("wpack", [128, COLS], F16, kind="ExternalInput")
    out_h = nc.dram_tensor("out", [R, NA], F32, kind="ExternalOutput")

    with tile.TileContext(nc) as tc:
        with (
            tc.tile_pool(name="consts", bufs=1) as consts,
            tc.tile_pool(name="persist", bufs=1) as persist,
            tc.tile_pool(name="work", bufs=2) as work,
            tc.tile_pool(name="ps", bufs=1, space="PSUM") as psp,
        ):
            wp = consts.tile([128, COLS], F16, tag="wp")

            def bias(i):
                return wp[:, C_BIAS + 2 * i:C_BIAS + 2 * i + 2].bitcast(F32)

            def wsl(c):
                return wp[:, c:c + 128]

            wp8 = wp[:, C_F8:C_F8 + NF8].bitcast(E4)
            pre8 = wp8[:, 0:256].rearrange("p (k m) -> p k m", k=2)
            lhi8 = wp8[:, 256:768].rearrange("p (k m) -> p k m", k=2)
            llo8 = wp8[:, 768:1280].rearrange("p (k m) -> p k m", k=2)

            locT = persist.tile([128, 2, NT, RT], F16, tag="locT")
            xsum = persist.tile([128, BS], F16, tag="xsum")
            g = persist.tile([128, 2, BS], F16, tag="g")
            gz = persist.tile([128, 4, BS], F32, tag="gz")
            qacc = persist.tile([128, 4 * NT, NA], F32, tag="qacc")

            # ---- DMAs ordered by first consumer; the head-critical ones go
            # through Pool SWDGE (tiny seq cost, bypasses the HWDGE queue) ----
            # obsT is one persistent [128, 2, NT*RT] region so multi-tile
            # DMAs land contiguously; obsTs[t] are views.
            obsT_all = persist.tile([128, 2, NT, RT], E4, tag="obsT")
            obsTs = [obsT_all[:, :, t, :] for t in range(NT)]
            nc.sync.dma_start(obsTs[0], obs_h[:, :, ts(0, RT)])
            nc.scalar.dma_start(wp[:, :C_WG0], wpack_h[:, :C_WG0])
            nc.sync.dma_start(obsTs[1], obs_h[:, :, ts(1, RT)])
            nc.sync.dma_start(obsT_all[:, :, 2:4, :], obs_h[:, :, 2 * RT:4 * RT])
            nc.sync.dma_start(wp[:, C_WG0:C_W1A], wpack_h[:, C_WG0:C_W1A])
            nc.sync.dma_start(wp[:, C_W1A:C_W2], wpack_h[:, C_W1A:C_W2])
            nc.sync.dma_start(obsT_all[:, :, 4:6, :], obs_h[:, :, 4 * RT:6 * RT])
            nc.sync.dma_start(wp[:, C_W2:C_W2 + 1024], wpack_h[:, C_W2:C_W2 + 1024])
            nc.sync.dma_start(obsT_all[:, :, 6:, :], obs_h[:, :, 6 * RT:])
            nc.sync.dma_start(wp[:, C_W2 + 1024:], wpack_h[:, C_W2 + 1024:])

            def ps_ax():
                return psp.tile([128, RT], F32, tag="ax", bufs=1, name="ax")

            def warmup():
                # Ramp the PE to full p-state during the head DMA wait:
                # dummy matmuls on scratch data keep the PE continuously
                # busy until obs arrives, so real matmuls run at 2.4 GHz.
                scrA = work.tile([128, 128], F16, tag="scrA", bufs=1,
                                 name="scrA")
                scrB = work.tile([128, RT], F16, tag="scrB", bufs=1,
                                 name="scrB")
                nc.vector.memset(scrA, 0.0)
                nc.gpsimd.memset(scrB, 0.0)
                dps = ps_ax()
                for _ in range(6):
                    nc.tensor.matmul(dps, scrA, scrB, start=True, stop=True)
                nc.vector.tensor_copy(scrB, dps)

            def ps_z1(nm="z1p"):
                return psp.tile([128, RT], F32, tag="z1p", bufs=3, name=nm)

            # ---- X(t): x-path for one tile -> xsum slice ----
            def X(t):
                if t < 4:
                    # preloop: z2p banks are free until iter 1
                    ps = psp.tile([128, 2, RT], F32, tag="z2p", bufs=2,
                                  name="xp2")[:, 0, :]
                elif t % 2 == 0:
                    ps = ps_ax()
                else:
                    ps = ps_z1("xp")
                nc.tensor.matmul(ps, pre8, obsTs[t], start=True, stop=True,
                                 perf_mode=DR)
                xT = work.tile([128, RT], F16, tag="xT", bufs=2, name="xT")
                nc.scalar.activation(xT, ps, RELU, bias=bias(B_PRE),
                                     scale=1.0 / WSCALE)
                with nc.allow_low_precision(reason="fp16 out; 16-elem sum"):
                    nc.vector.tensor_reduce(
                        xsum[:, ts(t, SPT)],
                        xT.rearrange("p (s k) -> p s k", k=N_AGENT),
                        axis=mybir.AxisListType.X,
                        op=AL.add,
                    )

            # ---- LOC(t): loc tile -> locT[:, :, t, :] ----
            def LOC(t):
                ohi_dup = obsTs[t][:, 0, :][:, None, :].to_broadcast(
                    [128, 2, RT])
                for m in range(2):
                    ps = ps_z1("locp")
                    nc.tensor.matmul(ps, lhi8[:, :, ts(m, 128)], obsTs[t],
                                     start=True, stop=False, perf_mode=DR)
                    nc.tensor.matmul(ps, llo8[:, :, ts(m, 128)], ohi_dup,
                                     start=False, stop=True, perf_mode=DR)
                    nc.scalar.activation(
                        locT[:, m, t, :], ps, RELU, bias=bias(B_LOC + m),
                        scale=1.0 / WSCALE)

            # ---- B steps over a sample slice (start s0, width w) ----
            def B_x1(s0, w):
                ps = ps_ax()
                nc.tensor.matmul(ps[:, :w], wsl(C_WG0), xsum[:, s0:s0 + w],
                                 start=True, stop=True)
                x1 = work.tile([128, 128], F16, tag="x1", bufs=2, name="x1")
                # 1/16 pre-folded into Wg0 on host so this runs on DVE
                nc.vector.tensor_scalar(x1[:, :w], ps[:, :w], bias(B_G0), 0.0,
                                        op0=AL.add, op1=AL.max)
                return x1[:, :w]

            def B_x2(x1, w):
                ps = ps_ax()
                nc.tensor.matmul(ps[:, :w], wsl(C_WG1), x1, start=True, stop=True)
                x2 = work.tile([128, 128], F16, tag="x2", bufs=2, name="x2")
                nc.vector.tensor_scalar(x2[:, :w], ps[:, :w], bias(B_G1), 0.0,
                                        op0=AL.add, op1=AL.max)
                return x2[:, :w]

            def B_g(x2, s0, w, early=False):
                ps = ps_z1("gp")
                for m in range(2):
                    nc.tensor.matmul(ps[:, ts(m, 256)][:, :w], wsl(C_WPOST + m * 128),
                                     x2, start=True, stop=True)
                    nc.vector.tensor_scalar(
                        g[:, m, s0:s0 + w], ps[:, ts(m, 256)][:, :w],
                        bias(B_POST + m), 0.0, op0=AL.add, op1=AL.max)

            def B_gz(s0, w, early=False):
                for mm in range(2):
                    ps = ps_z1("gzp")
                    for m in (2 * mm, 2 * mm + 1):
                        off = (m % 2) * 256
                        for kb in range(2):
                            nc.tensor.matmul(
                                ps[:, off:off + w],
                                wsl(C_W1A + kb * 512 + m * 128),
                                g[:, kb, s0:s0 + w],
                                start=(kb == 0), stop=(kb == 1),
                            )
                        nc.vector.tensor_scalar_add(
                            gz[:, m, s0:s0 + w], ps[:, off:off + w], bias(B_1 + m)
                        )

            # ---- C sections ----
            z1s = {}
            z2s = {}

            def z1_sec(t):
                z1 = work.tile([128, 4, RT], F16, tag="z1", bufs=3, name="z1")
                z1s[t] = z1
                for m in range(4):
                    ps = ps_z1()
                    for kb in range(2):
                        nc.tensor.matmul(
                            ps, wsl(C_W1B + kb * 512 + m * 128), locT[:, kb, t, :],
                            start=(kb == 0), stop=(kb == 1),
                        )
                    z1t = work.tile([128, RT], F16, tag="z1t", bufs=3, name="z1t")
                    nc.vector.tensor_add(
                        z1t.rearrange("p (s k) -> p s k", k=N_AGENT),
                        ps.rearrange("p (s k) -> p s k", k=N_AGENT),
                        gz[:, m, ts(t, SPT)][:, :, None].to_broadcast(
                            [128, SPT, N_AGENT]),
                    )
                    # last tile: keep DVE clear for the z2(7) evacuations
                    if m % 2 == 0 and t < 7:
                        nc.vector.tensor_scalar(z1[:, m, :], z1t, 0.0, 0.0,
                                                op0=AL.add, op1=AL.max)
                    else:
                        nc.gpsimd.tensor_scalar(z1[:, m, :], z1t, 0.0, 0.0,
                                                op0=AL.add, op1=AL.max)

            def z2_sec(t, spread=False):
                z1 = z1s.pop(t)
                z2 = work.tile([128, 4, RT], F16, tag="z2", bufs=2, name="z2")
                z2s[t] = z2
                pss = [psp.tile([128, 2, RT], F32, tag="z2p", bufs=2, name="z2p")
                       for _ in range(2)]
                for kb in range(4):
                    for m in range(4):
                        nc.tensor.matmul(
                            pss[m // 2][:, m % 2, :],
                            wsl(C_W2 + m * 512 + kb * 128), z1[:, kb, :],
                            start=(kb == 0), stop=(kb == 3),
                        )
                for m in range(4):
                    src = pss[m // 2][:, m % 2, :]
                    if spread and m % 2 == 1:
                        nc.vector.tensor_scalar(z2[:, m, :], src, bias(B_2 + m),
                                                0.0, op0=AL.add, op1=AL.max)
                    else:
                        nc.scalar.activation(z2[:, m, :], src, RELU,
                                             bias=bias(B_2 + m))

            def q_sec(t):
                z2 = z2s.pop(t)
                qp = ps_z1("qp") if t == 7 else ps_ax()
                qv = qp[:, :4 * NA].rearrange("p (nb a) -> p nb a", a=NA)
                for nb in range(4):
                    for kb in range(4):
                        nc.tensor.matmul(
                            qv[:, nb, :], z2[:, kb, ts(nb, 128)],
                            wp[:, C_W3 + kb * 8:C_W3 + (kb + 1) * 8],
                            start=(kb == 0), stop=(kb == 3),
                        )
                nc.vector.tensor_copy(qacc[:, ts(t, 4), :], qv)
                if t % 2 == 1 and t < 6:
                    qt = t // 2
                    nc.sync.dma_start(
                        out_h[ts(qt, 2 * RT), :].rearrange("(nb p) a -> p nb a",
                                                           p=128),
                        qacc[:, ts(qt, 8), :],
                    )
                elif t >= 6:
                    nc.sync.dma_start(
                        out_h[ts(t, RT), :].rearrange("(nb p) a -> p nb a", p=128),
                        qacc[:, ts(t, 4), :],
                    )

            # ---- schedule ----
            warmup()
            # tile-0 B chain first (32 samples) so C can start early
            X(0)
            X(1)
            LOC(0)
            LOC(1)
            x1 = B_x1(0, SPT)
            X(2)
            x2 = B_x2(x1, SPT)
            X(3)
            B_g(x2, 0, SPT, early=True)
            B_gz(0, SPT, early=True)
            x1 = B_x1(SPT, SPT)
            # iter 0
            z1_sec(0)
            x2 = B_x2(x1, SPT)
            LOC(2)
            B_g(x2, SPT, SPT, early=True)
            X(4)
            B_gz(SPT, SPT, early=True)
            x1 = B_x1(64, 64)
            # iter 1
            z2_sec(0)
            x2 = B_x2(x1, 64)
            z1_sec(1); LOC(3); X(5); X(6)
            B_g(x2, 64, 64); B_gz(64, 64)
            # iter 2
            z2_sec(1); z1_sec(2); LOC(4); q_sec(0); X(7)
            x1 = B_x1(128, 128)
            # iter 3
            z2_sec(2)
            x2 = B_x2(x1, 128)
            z1_sec(3); LOC(5); q_sec(1)
            B_g(x2, 128, 128); B_gz(128, 128)
            # iter 4
            z2_sec(3); z1_sec(4); LOC(6); q_sec(2)
            # iter 5
            z2_sec(4); z1_sec(5); LOC(7); q_sec(3)
            # iter 6
            z2_sec(5); z1_sec(6); q_sec(4)
            # iter 7 (z2(6) evacs stay on ACT: DVE must stay clear
            # for z1(7)'s adds, which gate the flush z2(7) matmuls)
            z2_sec(6); z1_sec(7); q_sec(5)
            # flush
            z2_sec(7, spread=True); q_sec(6); q_sec(7)

    _split_waits(nc)
    return nc


def _split_waits(nc):
    # walrus accepts only one sync-wait per instruction in this build; move
    # extra waits onto same-engine sequencer nops placed immediately before
    # the instruction (program order on the engine's queue, so semantics are
    # identical).
    for blk in nc.m.functions[0].blocks:
        new = []
        for inst in blk.instructions:
            if inst.sync_info is not None:
                w = list(inst.sync_info.on_wait)
                if len(w) > 1:
                    for wx in w[:-1]:
                        new.append(
                            mybir.InstNoOp(
                                name=nc.get_next_instruction_name(),
                                engine=inst.engine,
                                sync_info=mybir.SyncInfo(
                                    on_wait=[wx], on_update=[]
                                ),
                                bass_nofuse=True,
                            )
                        )
                    inst.sync_info.on_wait = [w[-1]]
            new.append(inst)
        blk.instructions[:] = new


_CACHE = {}


def _get_nc():
    if "nc" not in _CACHE:
        _CACHE["nc"] = _build()
    return _CACHE["nc"]


def kernel(trace=False, **inputs):
    obs_j = np.ascontiguousarray(np.asarray(inputs["obs_j"], dtype=np.float32))
    np_in = {
        k: np.asarray(v, dtype=np.float32)
        for k, v in inputs.items()
        if k != "obs_j"
    }
    pack = np.ascontiguousarray(_pack_weights(np_in))
    nc = _get_nc()
    in_maps = []
    for c in range(NCORES):
        obsT = obs_j[c * BS:(c + 1) * BS].reshape(R, OBS).T
        ohi = obsT.astype(FP8)
        olo = (obsT - ohi.astype(np.float32)).astype(FP8)
        in_maps.append({
            "obs": np.ascontiguousarray(np.stack([ohi, olo], axis=1)),
            "wpack": pack,
        })
    res = run_bass_kernel_spmd(
        nc, in_maps, core_ids=list(range(NCORES)), trace=trace
    )
    out = np.concatenate([r["out"] for r in res.results], axis=0)  # [B*n, NA]
    q = out.reshape(B, N_AGENT, NA) + np_in["b3"]
    if trace:
        return q, res
    return q


# revision 7
# speedup vs baseline: 1.0337x; 1.0057x over previous
"""GCNCritic forward kernel for Trainium2 (Bass/Tile), 8-core data-parallel.

Math collapse (same as baseline): fully-connected 16-node graphs with
self-loops make each GCN layer's output constant across a sample's nodes, so
the GCN stack reduces to per-sample matmuls on the per-sample sum of
x = relu(obs @ W_pre + b_pre):

    xsum = sum_nodes(relu(obs @ W_pre + b_pre))           # [B, HID]
    x1 = relu(xsum/16 @ W_gcn0 + b_gcn0)                  # [B, HID]
    x2 = relu(x1 @ W_gcn1 + b_gcn1)                       # [B, HID]
    g  = relu(x2 @ W_post + b_post)                       # [B, GE]
    gz = g @ W1[:GE] + b1                                 # [B, F1]
    loc = relu(obs @ W_loc + b_loc)                       # [B*n, LE]
    z1 = relu(loc @ W1[GE:] + gz[sample])                 # [B*n, F1]
    z2 = relu(z1 @ W2 + b2)                               # [B*n, F2]
    q  = z2 @ W3  (+ b3 added on host)                    # [B*n, 8]

v2 vs baseline (73929 ns -> 64915 ns in the cost-model timeline):
  * Matmul operands fp16 (same 1 cyc/row PE rate as float32r here, but
    halves obs+weight DMA; rel err ~1e-3 << 2e-2).
  * obs ships as an fp8e4 hi/lo pair (same bytes as fp16, exact to ~2^-8):
    x = obs @ W_pre runs as ONE DoubleRow fp8 matmul (half cost) with W_pre
    quantized fp8 (*64, rescaled in the evacuation); loc = obs @ W_loc runs
    as two DoubleRow matmuls with W_loc split hi+lo (weights exact), so the
    only fp8 quantization error is W_pre (~4e-3 total, gate is 2e-2).
  * q = z2 @ W3 operand-swapped: z2^T node-blocks stationary, W3 [128,8]
    streams -> 8-row matmuls (16384 -> 1024 PE cycles) and node-major
    output [R, 8], the natural layout (b3 added on host).
  * Software-pipelined C phase: z2(t-1) runs kb-major so it consumes
    z1(t-1) m-blocks as their DVE add+relu evacuations land; q(t-2) and
    the deferred loc/x tiles fill remaining PE slots. GPSIMD cannot touch
    PSUM, so PSUM evacuations live on ACT+DVE only; Pool gets the
    SBUF-only z1 relus.
  * PE p-state warmup: dummy matmuls during the head DMA wait ramp the
    PE to 2.4 GHz before real work arrives.
  * DMAs ordered by first consumer, obs grouped to cut HWDGE/SP-issue
    serialization; output DMA split per 2 tiles to shorten the tail.
"""

import ml_dtypes
import numpy as np

import concourse.bass as bass
import concourse.mybir as mybir
import concourse.tile as tile
from concourse.bass import ts
from concourse.bass_utils import run_bass_kernel_spmd

OBS = 128
N_AGENT = 16
HID = 128
GE = 256
LE = 256
F1 = 512
F2 = 512
NA = 8
B = 2048
NCORES = 8
BS = B // NCORES            # 256 samples per core
R = BS * N_AGENT            # 4096 rows (nodes) per core
RT = 512                    # rows per tile
NT = R // RT                # 8 row tiles
SPT = RT // N_AGENT         # 32 samples per row tile

F32 = mybir.dt.float32
F16 = mybir.dt.float16
E4 = mybir.dt.float8e4
FP8 = ml_dtypes.float8_e4m3
DR = mybir.MatmulPerfMode.DoubleRow
RELU = mybir.ActivationFunctionType.Relu
AL = mybir.AluOpType

NB_F32 = 15
C_BIAS = 0
# fp8 section (stored as fp16 columns, bitcast to fp8e4 on device):
# [0:256)    W_pre*64 duplicated over ko            (x DoubleRow lhsT)
# [256:768)  W_loc_hi*64 dup over ko, m-major       (loc DR term 1)
# [768:1280) W_loc_lo*64 (ko=0) + zeros (ko=1)      (loc DR term 2)
C_F8 = 2 * NB_F32            # 30
NF8 = 640                    # fp16 cols = 1280 fp8 cols
WSCALE = 64.0
C_WG0 = C_F8 + NF8           # 670
C_WG1 = C_WG0 + 128          # 798
C_WPOST = C_WG1 + 128        # 926
C_W1A = C_WPOST + 256        # 1182  (kb-major: kb*512 + m*128)
C_W1B = C_W1A + 1024         # 2206
C_W2 = C_W1B + 1024          # 3230  (m-major: m*512 + kb*128)
C_W3 = C_W2 + 2048           # 5278  (kb*8)
COLS = C_W3 + 32             # 5310

B_PRE = 0
B_G0 = 1
B_G1 = 2
B_POST = 3
B_LOC = 5
B_1 = 7
B_2 = 11


def _pack_weights(i):
    pk = np.zeros((128, COLS), np.float16)
    f16 = lambda a: np.asarray(a, np.float32).astype(np.float16)
    f8 = np.zeros((128, 2 * NF8), FP8)
    wpre8 = (np.asarray(i["W_pre"], np.float32) * WSCALE).astype(FP8)
    f8[:, 0:128] = wpre8
    f8[:, 128:256] = wpre8
    wloc = np.asarray(i["W_loc"], np.float32) * WSCALE
    whi = wloc.astype(FP8)
    wlo = (wloc - whi.astype(np.float32)).astype(FP8)
    f8[:, 256:512] = whi
    f8[:, 512:768] = whi
    f8[:, 768:1024] = wlo
    pk[:, C_F8:C_F8 + NF8] = f8.view(np.float16)
    pk[:, C_WG0:C_WG0 + 128] = f16(np.asarray(i["W_gcn"][0]) / N_AGENT)
    pk[:, C_WG1:C_WG1 + 128] = f16(i["W_gcn"][1])
    pk[:, C_WPOST:C_WPOST + 256] = f16(i["W_post"])
    for kb in range(2):
        for m in range(4):
            pk[:, C_W1A + kb * 512 + m * 128:C_W1A + kb * 512 + (m + 1) * 128] = \
                f16(i["W1"][kb * 128:(kb + 1) * 128, m * 128:(m + 1) * 128])
            pk[:, C_W1B + kb * 512 + m * 128:C_W1B + kb * 512 + (m + 1) * 128] = \
                f16(i["W1"][GE + kb * 128:GE + (kb + 1) * 128, m * 128:(m + 1) * 128])
    for kb in range(4):
        for m in range(4):
            pk[:, C_W2 + m * 512 + kb * 128:C_W2 + m * 512 + (kb + 1) * 128] = \
                f16(i["W2"][kb * 128:(kb + 1) * 128, m * 128:(m + 1) * 128])
        pk[:, C_W3 + kb * 8:C_W3 + (kb + 1) * 8] = f16(i["W3"][kb * 128:(kb + 1) * 128])
    bias = np.zeros((128, NB_F32), np.float32)
    bias[:, B_PRE] = i["b_pre"]
    bias[:, B_G0] = i["b_gcn"][0]
    bias[:, B_G1] = i["b_gcn"][1]
    bias[:, B_POST:B_POST + 2] = i["b_post"].reshape(2, 128).T
    bias[:, B_LOC:B_LOC + 2] = i["b_loc"].reshape(2, 128).T
    bias[:, B_1:B_1 + 4] = i["b1"].reshape(4, 128).T
    bias[:, B_2:B_2 + 4] = i["b2"].reshape(4, 128).T
    pk[:, C_BIAS:C_BIAS + 2 * NB_F32] = bias.view(np.float16)
    return pk


def _build():
    nc = bass.Bass("TRN2", target_bir_lowering=False, debug=False)

    obs_h = nc.dram_tensor("obs", [OBS, 2, R], E4, kind="ExternalInput")
    wpack_h = nc.dram_tensor("wpack", [128, COLS], F16, kind="ExternalInput")
    out_h = nc.dram_tensor("out", [R, NA], F32, kind="ExternalOutput")

    with tile.TileContext(nc) as tc:
        with (
            tc.tile_pool(name="consts", bufs=1) as consts,
            tc.tile_pool(name="persist", bufs=1) as persist,
            tc.tile_pool(name="work", bufs=2) as work,
            tc.tile_pool(name="ps", bufs=1, space="PSUM") as psp,
        ):
            wp = consts.tile([128, COLS], F16, tag="wp")

            def bias(i):
                return wp[:, C_BIAS + 2 * i:C_BIAS + 2 * i + 2].bitcast(F32)

            def wsl(c):
                return wp[:, c:c + 128]

            wp8 = wp[:, C_F8:C_F8 + NF8].bitcast(E4)
            pre8 = wp8[:, 0:256].rearrange("p (k m) -> p k m", k=2)
            lhi8 = wp8[:, 256:768].rearrange("p (k m) -> p k m", k=2)
            llo8 = wp8[:, 768:1280].rearrange("p (k m) -> p k m", k=2)

            locT = persist.tile([128, 2, NT, RT], F16, tag="locT")
            xsum = persist.tile([128, BS], F16, tag="xsum")
            g = persist.tile([128, 2, BS], F16, tag="g")
            gz = persist.tile([128, 4, BS], F32, tag="gz")
            qacc = persist.tile([128, 4 * NT, NA], F32, tag="qacc")

            # ---- DMAs ordered by first consumer; the head-critical ones go
            # through Pool SWDGE (tiny seq cost, bypasses the HWDGE queue) ----
            # obsT is one persistent [128, 2, NT*RT] region so multi-tile
            # DMAs land contiguously; obsTs[t] are views.
            obsT_all = persist.tile([128, 2, NT, RT], E4, tag="obsT")
            obsTs = [obsT_all[:, :, t, :] for t in range(NT)]
            nc.sync.dma_start(obsTs[0], obs_h[:, :, ts(0, RT)])
            nc.scalar.dma_start(wp[:, :C_WG0], wpack_h[:, :C_WG0])
            nc.sync.dma_start(obsTs[1], obs_h[:, :, ts(1, RT)])
            nc.sync.dma_start(obsT_all[:, :, 2:4, :], obs_h[:, :, 2 * RT:4 * RT])
            nc.sync.dma_start(wp[:, C_WG0:C_W1A], wpack_h[:, C_WG0:C_W1A])
            nc.sync.dma_start(wp[:, C_W1A:C_W2], wpack_h[:, C_W1A:C_W2])
            nc.sync.dma_start(obsT_all[:, :, 4:6, :], obs_h[:, :, 4 * RT:6 * RT])
            nc.sync.dma_start(wp[:, C_W2:C_W2 + 1024], wpack_h[:, C_W2:C_W2 + 1024])
            nc.sync.dma_start(obsT_all[:, :, 6:, :], obs_h[:, :, 6 * RT:])
            nc.sync.dma_start(wp[:, C_W2 + 1024:], wpack_h[:, C_W2 + 1024:])

            def ps_ax():
                return psp.tile([128, RT], F32, tag="ax", bufs=1, name="ax")

            def warmup():
                # Ramp the PE to full p-state during the head DMA wait:
                # dummy matmuls on scratch data keep the PE continuously
                # busy until obs arrives, so real matmuls run at 2.4 GHz.
                scrA = work.tile([128, 128], F16, tag="scrA", bufs=1,
                                 name="scrA")
                scrB = work.tile([128, RT], F16, tag="scrB", bufs=1,
                                 name="scrB")
                nc.vector.memset(scrA, 0.0)
                nc.gpsimd.memset(scrB, 0.0)
                dps = ps_ax()
                for _ in range(6):
                    nc.tensor.matmul(dps, scrA, scrB, start=True, stop=True)
                nc.vector.tensor_copy(scrB, dps)

            def ps_z1(nm="z1p"):
                return psp.tile([128, RT], F32, tag="z1p", bufs=3, name=nm)

            # ---- X(t): x-path for one tile -> xsum slice ----
            def X(t):
                if t < 4:
                    # preloop: z2p banks are free until iter 1
                    ps = psp.tile([128, 2, RT], F32, tag="z2p", bufs=2,
                                  name="xp2")[:, 0, :]
                elif t % 2 == 0:
                    ps = ps_ax()
                else:
                    ps = ps_z1("xp")
                nc.tensor.matmul(ps, pre8, obsTs[t], start=True, stop=True,
                                 perf_mode=DR)
                xT = work.tile([128, RT], F16, tag="xT", bufs=2, name="xT")
                nc.scalar.activation(xT, ps, RELU, bias=bias(B_PRE),
                                     scale=1.0 / WSCALE)
                with nc.allow_low_precision(reason="fp16 out; 16-elem sum"):
                    nc.vector.tensor_reduce(
                        xsum[:, ts(t, SPT)],
                        xT.rearrange("p (s k) -> p s k", k=N_AGENT),
                        axis=mybir.AxisListType.X,
                        op=AL.add,
                    )

            # ---- LOC(t): loc tile -> locT[:, :, t, :] ----
            def LOC(t):
                ohi_dup = obsTs[t][:, 0, :][:, None, :].to_broadcast(
                    [128, 2, RT])
                for m in range(2):
                    ps = ps_z1("locp")
                    nc.tensor.matmul(ps, lhi8[:, :, ts(m, 128)], obsTs[t],
                                     start=True, stop=False, perf_mode=DR)
                    nc.tensor.matmul(ps, llo8[:, :, ts(m, 128)], ohi_dup,
                                     start=False, stop=True, perf_mode=DR)
                    nc.scalar.activation(
                        locT[:, m, t, :], ps, RELU, bias=bias(B_LOC + m),
                        scale=1.0 / WSCALE)

            # ---- B steps over a sample slice (start s0, width w) ----
            def B_x1(s0, w):
                ps = ps_ax()
                nc.tensor.matmul(ps[:, :w], wsl(C_WG0), xsum[:, s0:s0 + w],
                                 start=True, stop=True)
                x1 = work.tile([128, 128], F16, tag="x1", bufs=2, name="x1")
                # 1/16 pre-folded into Wg0 on host so this runs on DVE
                nc.vector.tensor_scalar(x1[:, :w], ps[:, :w], bias(B_G0), 0.0,
                                        op0=AL.add, op1=AL.max)
                return x1[:, :w]

            def B_x2(x1, w):
                ps = ps_ax()
                nc.tensor.matmul(ps[:, :w], wsl(C_WG1), x1, start=True, stop=True)
                x2 = work.tile([128, 128], F16, tag="x2", bufs=2, name="x2")
                nc.vector.tensor_scalar(x2[:, :w], ps[:, :w], bias(B_G1), 0.0,
                                        op0=AL.add, op1=AL.max)
                return x2[:, :w]

            def B_g(x2, s0, w, early=False):
                ps = ps_z1("gp")
                for m in range(2):
                    nc.tensor.matmul(ps[:, ts(m, 256)][:, :w], wsl(C_WPOST + m * 128),
                                     x2, start=True, stop=True)
                    nc.vector.tensor_scalar(
                        g[:, m, s0:s0 + w], ps[:, ts(m, 256)][:, :w],
                        bias(B_POST + m), 0.0, op0=AL.add, op1=AL.max)

            def B_gz(s0, w, early=False):
                for mm in range(2):
                    ps = ps_z1("gzp")
                    for m in (2 * mm, 2 * mm + 1):
                        off = (m % 2) * 256
                        for kb in range(2):
                            nc.tensor.matmul(
                                ps[:, off:off + w],
                                wsl(C_W1A + kb * 512 + m * 128),
                                g[:, kb, s0:s0 + w],
                                start=(kb == 0), stop=(kb == 1),
                            )
                        nc.vector.tensor_scalar_add(
                            gz[:, m, s0:s0 + w], ps[:, off:off + w], bias(B_1 + m)
                        )

            # ---- C sections ----
            z1s = {}
            z2s = {}

            def z1_sec(t, pre=False):
                z1 = work.tile([128, 4, RT], F16, tag="z1", bufs=3, name="z1")
                z1s[t] = z1
                for m in range(4):
                    # tile 0: z2p banks are free until iter 1 -- using them
                    # keeps z1(0) off the z1p rotation crowded by the B chain
                    ps = psp.tile([128, 2, RT], F32, tag="z2p", bufs=2,
                                  name="z1p0")[:, 0, :] if pre else ps_z1()
                    for kb in range(2):
                        nc.tensor.matmul(
                            ps, wsl(C_W1B + kb * 512 + m * 128), locT[:, kb, t, :],
                            start=(kb == 0), stop=(kb == 1),
                        )
                    z1t = work.tile([128, RT], F16, tag="z1t", bufs=3, name="z1t")
                    nc.vector.tensor_add(
                        z1t.rearrange("p (s k) -> p s k", k=N_AGENT),
                        ps.rearrange("p (s k) -> p s k", k=N_AGENT),
                        gz[:, m, ts(t, SPT)][:, :, None].to_broadcast(
                            [128, SPT, N_AGENT]),
                    )
                    # last tile: keep DVE clear for the z2(7) evacuations
                    if m % 2 == 0 and t < 7:
                        nc.vector.tensor_scalar(z1[:, m, :], z1t, 0.0, 0.0,
                                                op0=AL.add, op1=AL.max)
                    else:
                        nc.gpsimd.tensor_scalar(z1[:, m, :], z1t, 0.0, 0.0,
                                                op0=AL.add, op1=AL.max)

            def z2_sec(t, spread=False):
                z1 = z1s.pop(t)
                z2 = work.tile([128, 4, RT], F16, tag="z2", bufs=2, name="z2")
                z2s[t] = z2
                pss = [psp.tile([128, 2, RT], F32, tag="z2p", bufs=2, name="z2p")
                       for _ in range(2)]
                for kb in range(4):
                    for m in range(4):
                        nc.tensor.matmul(
                            pss[m // 2][:, m % 2, :],
                            wsl(C_W2 + m * 512 + kb * 128), z1[:, kb, :],
                            start=(kb == 0), stop=(kb == 3),
                        )
                for m in range(4):
                    src = pss[m // 2][:, m % 2, :]
                    if spread and m % 2 == 1:
                        nc.vector.tensor_scalar(z2[:, m, :], src, bias(B_2 + m),
                                                0.0, op0=AL.add, op1=AL.max)
                    else:
                        nc.scalar.activation(z2[:, m, :], src, RELU,
                                             bias=bias(B_2 + m))

            def q_sec(t):
                z2 = z2s.pop(t)
                qp = ps_z1("qp") if t == 7 else ps_ax()
                qv = qp[:, :4 * NA].rearrange("p (nb a) -> p nb a", a=NA)
                for nb in range(4):
                    for kb in range(4):
                        nc.tensor.matmul(
                            qv[:, nb, :], z2[:, kb, ts(nb, 128)],
                            wp[:, C_W3 + kb * 8:C_W3 + (kb + 1) * 8],
                            start=(kb == 0), stop=(kb == 3),
                        )
                nc.vector.tensor_copy(qacc[:, ts(t, 4), :], qv)
                if t % 2 == 1 and t < 6:
                    qt = t // 2
                    nc.sync.dma_start(
                        out_h[ts(qt, 2 * RT), :].rearrange("(nb p) a -> p nb a",
                                                           p=128),
                        qacc[:, ts(qt, 8), :],
                    )
                elif t >= 6:
                    nc.sync.dma_start(
                        out_h[ts(t, RT), :].rearrange("(nb p) a -> p nb a", p=128),
                        qacc[:, ts(t, 4), :],
                    )

            # ---- schedule ----
            warmup()
            # tile-0 B chain first (32 samples) so C can start early
            X(0)
            X(1)
            X(2)
            X(3)
            LOC(0)
            x1 = B_x1(0, SPT)
            LOC(1)
            x2 = B_x2(x1, SPT)
            B_g(x2, 0, SPT, early=True)
            B_gz(0, SPT, early=True)
            x1 = B_x1(SPT, SPT)
            # iter 0
            z1_sec(0, pre=True)
            x2 = B_x2(x1, SPT)
            LOC(2)
            B_g(x2, SPT, SPT, early=True)
            X(4)
            B_gz(SPT, SPT, early=True)
            x1 = B_x1(64, 64)
            # iter 1
            z2_sec(0)
            x2 = B_x2(x1, 64)
            z1_sec(1); LOC(3); X(5); X(6)
            B_g(x2, 64, 64); B_gz(64, 64)
            # iter 2
            z2_sec(1); z1_sec(2); LOC(4); q_sec(0); X(7)
            x1 = B_x1(128, 128)
            # iter 3
            z2_sec(2)
            x2 = B_x2(x1, 128)
            z1_sec(3); LOC(5); q_sec(1)
            B_g(x2, 128, 128); B_gz(128, 128)
            # iter 4
            z2_sec(3); z1_sec(4); LOC(6); q_sec(2)
            # iter 5
            z2_sec(4); z1_sec(5); LOC(7); q_sec(3)
            # iter 6
            z2_sec(5); z1_sec(6); q_sec(4)
            # iter 7 (z2(6) evacs stay on ACT: DVE must stay clear
            # for z1(7)'s adds, which gate the flush z2(7) matmuls)
            z2_sec(6); z1_sec(7); q_sec(5)
            # flush
            z2_sec(7, spread=True); q_sec(6); q_sec(7)

    _split_waits(nc)
    return nc


def _split_waits(nc):
    # walrus accepts only one sync-wait per instruction in this build; move
    # extra waits onto same-engine sequencer nops placed immediately before
    # the instruction (program order on the engine's queue, so semantics are
    # identical).
    for blk in nc.m.functions[0].blocks:
        new = []
        for inst in blk.instructions:
            if inst.sync_info is not None:
                w = list(inst.sync_info.on_wait)
                if len(w) > 1:
                    for wx in w[:-1]:
                        new.append(
                            mybir.InstNoOp(
                                name=nc.get_next_instruction_name(),
                                engine=inst.engine,
                                sync_info=mybir.SyncInfo(
                                    on_wait=[wx], on_update=[]
                                ),
                                bass_nofuse=True,
                            )
                        )
                    inst.sync_info.on_wait = [w[-1]]
            new.append(inst)
        blk.instructions[:] = new


_CACHE = {}


def _get_nc():
    if "nc" not in _CACHE:
        _CACHE["nc"] = _build()
    return _CACHE["nc"]


def kernel(trace=False, **inputs):
    obs_j = np.ascontiguousarray(np.asarray(inputs["obs_j"], dtype=np.float32))
    np_in = {
        k: np.asarray(v, dtype=np.float32)
        for k, v in inputs.items()
        if k != "obs_j"
    }
    pack = np.ascontiguousarray(_pack_weights(np_in))
    nc = _get_nc()
    in_maps = []
    for c in range(NCORES):
        obsT = obs_j[c * BS:(c + 1) * BS].reshape(R, OBS).T
        ohi = obsT.astype(FP8)
        olo = (obsT - ohi.astype(np.float32)).astype(FP8)
        in_maps.append({
            "obs": np.ascontiguousarray(np.stack([ohi, olo], axis=1)),
            "wpack": pack,
        })
    res = run_bass_kernel_spmd(
        nc, in_maps, core_ids=list(range(NCORES)), trace=trace
    )
    out = np.concatenate([r["out"] for r in res.results], axis=0)  # [B*n, NA]
    q = out.reshape(B, N_AGENT, NA) + np_in["b3"]
    if trace:
        return q, res
    return q
